# revision 9
# baseline (speedup 1.0000x reference)
"""NeuralRetriever kernel for 8x Trainium2 NeuronCores (Bass/Tile).

Full inputs in, full outputs out. Internally: doc_embeddings sharded along
num_docs across 8 cores; each core mean-pools + encodes its shard, scores all
64 queries against it, and returns a local top-8 (values + indices) per
query-half-row. Host merges the 8*2*8 candidates per query and re-top-ks.

reference semantics (see problem):
    q = mlp_ln(mean_t(query_embeddings)); d = mlp_ln(mean_t(doc_embeddings))
    qn = l2norm(q @ pW + pb); dn = l2norm(d @ pW + pb)
    scores = qn @ dn.T / temperature; top8 + gathered docs.
"""

import os

import numpy as np

import concourse.bass as bass
import concourse.mybir as mybir
from concourse import bacc
from concourse.tile import TileContext
from concourse.bass_utils import run_bass_kernel_spmd
from concourse.masks import make_identity

F32 = mybir.dt.float32
U32 = mybir.dt.uint32
AF = mybir.ActivationFunctionType
AX = mybir.AxisListType

N_CORES = 8
N_DOCS = 50000
DPC = N_DOCS // N_CORES          # 6250 docs per core
D_TOK = 16
Q_TOK = 32
EMB = 768
HID = 512
NQ = 64
LN_EPS = 1e-5

BLK = 128                         # pooling block (docs)
MAC = 512                         # matmul macro-block (docs)
HI_OFF = 3072                     # docs >= HI_OFF go to partition 64+q
HI_W = DPC - HI_OFF               # 3178 columns in the hi half
KC1 = EMB // 128                  # 6  K-chunks for layer 1
KC2 = HID // 128                  # 4  K-chunks for hid-dim contractions
MC = HID // 128                   # 4  M-chunks of hidden dim

# pooling blocks: 48 aligned + 1 overlapping tail covering [6122, 6250)
BLOCK_STARTS = [i * BLK for i in range(DPC // BLK)] + [DPC - BLK]
# macros: groups of 4 blocks (512 docs) + tail macro of 1 block
MACROS = [list(range(i, i + 4)) for i in range(0, 48, 4)] + [[48]]


def _mlp_weights(tc, sb, nc, w1_d, w2_d, pw_d, g_d, tok_div, tmp_sb=None):
    """Load + prep encoder weights into SBUF lhsT layouts.

    Returns (w1s [128, KC1*512] scaled by 1/tok_div, w2s [128, KC2*512],
             pwg [128, KC2*512] = pW * g[k] rows)."""
    w1s = sb.tile([128, KC1 * HID], F32)
    nc.sync.dma_start(out=w1s.rearrange("p (k m) -> p k m", k=KC1),
                      in_=w1_d[:, :, :].rearrange("k p m -> p k m"))
    nc.vector.tensor_scalar_mul(w1s, w1s, 1.0 / tok_div)
    w2s = sb.tile([128, KC2 * HID], F32)
    nc.sync.dma_start(out=w2s.rearrange("p (k m) -> p k m", k=KC2),
                      in_=w2_d[:, :, :].rearrange("k p m -> p k m"))
    g_t = sb.tile([128, KC2], F32)
    nc.sync.dma_start(out=g_t, in_=g_d[:, :].rearrange("c p -> p c"))
    pw_t = (tmp_sb or sb).tile([128, KC2 * HID], F32, name=f"pwt_{id(w1_d)}")
    nc.sync.dma_start(out=pw_t.rearrange("p (k m) -> p k m", k=KC2),
                      in_=pw_d[:, :, :].rearrange("k p m -> p k m"))
    pwg = sb.tile([128, KC2 * HID], F32)
    for kc in range(KC2):
        nc.vector.tensor_scalar(out=pwg[:, kc * HID:(kc + 1) * HID],
                                in0=pw_t[:, kc * HID:(kc + 1) * HID],
                                scalar1=g_t[:, kc:kc + 1], scalar2=None,
                                op0=mybir.AluOpType.mult)
    return w1s, w2s, pw_t, pwg


def _proj_consts(tc, sb, nc, pw_t, pwg, beta_t, pb_row, ones_col, tag):
    """c_neg [1,512] = -colsum(pwg);  vrow [1,512] = beta^T pW + pb."""
    ctx = tc.tile_pool(name=f"{tag}_cps", bufs=1, space="PSUM")
    ps = ctx.__enter__()
    c_ps = ps.tile([1, HID], F32)
    for kc in range(KC2):
        nc.tensor.matmul(c_ps, ones_col, pwg[:, kc * HID:(kc + 1) * HID],
                         start=(kc == 0), stop=(kc == KC2 - 1))
    c_neg = sb.tile([1, HID], F32)
    nc.scalar.mul(c_neg, c_ps, -1.0)
    v_ps = ps.tile([1, HID], F32)
    for kc in range(KC2):
        nc.tensor.matmul(v_ps, beta_t[:, kc:kc + 1],
                         pw_t[:, kc * HID:(kc + 1) * HID],
                         start=(kc == 0), stop=(kc == KC2 - 1))
    vrow = sb.tile([1, HID], F32)
    nc.vector.tensor_add(vrow, pb_row, v_ps)
    ctx.__exit__(None, None, None)
    return c_neg, vrow


def _encode(tc, sb, ps, nc, xT, n, w1s, w2s, pwg, b1_t, b2_t, c_neg, vrow,
            ones_inv, ones_col, eps_t, zero_t, tag):
    """Feature-major MLP + LN + proj with LN scale folded (B = A + u*v).

    xT: SBUF tile [128, KC1*ncols] feature-major pooled input (token SUM; the
        1/T is folded into w1s). Chunk kc occupies cols [kc*ncols, kc*ncols+n).
    Returns (zd [128, KC2*ncols] SBUF feature-major un-normalized projected
             reps, ninv [1, n] SBUF 1/||B|| per doc).
    ncols is the allocation stride (>= n)."""
    ncols = xT.shape[1] // KC1
    h1 = sb.tile([128, KC2 * ncols], F32, tag=f"{tag}_h1", bufs=1)
    for mc in range(MC):
        h1_ps = ps.tile([128, n], F32, tag=f"{tag}_h1ps", bufs=1)
        for kc in range(KC1):
            nc.tensor.matmul(h1_ps,
                             w1s[:, kc * HID + mc * 128: kc * HID + (mc + 1) * 128],
                             xT[:, kc * ncols: kc * ncols + n],
                             start=(kc == 0), stop=(kc == KC1 - 1))
        nc.scalar.activation(h1[:, mc * ncols: mc * ncols + n], h1_ps,
                             AF.Relu, bias=b1_t[:, mc:mc + 1], scale=1.0)
    h2 = sb.tile([128, KC2 * ncols], F32, tag=f"{tag}_h2", bufs=1)
    h2sq = sb.tile([128, KC2 * ncols], F32, tag=f"{tag}_h2sq", bufs=1)
    stats = ps.tile([65, max(n, 16)], F32, tag=f"{tag}_stats", bufs=1)
    for mc in range(MC):
        h2_ps = ps.tile([128, n], F32, tag=f"{tag}_h2ps", bufs=1)
        for kc in range(KC2):
            nc.tensor.matmul(h2_ps,
                             w2s[:, kc * HID + mc * 128: kc * HID + (mc + 1) * 128],
                             h1[:, kc * ncols: kc * ncols + n],
                             start=(kc == 0), stop=(kc == KC2 - 1))
        nc.scalar.activation(h2[:, mc * ncols: mc * ncols + n], h2_ps,
                             AF.Identity, bias=b2_t[:, mc:mc + 1], scale=1.0)
        nc.scalar.activation(h2sq[:, mc * ncols: mc * ncols + n], h2_ps,
                             AF.Square, bias=b2_t[:, mc:mc + 1], scale=1.0)
    # mu (row 0) and E[h^2] (row 1), each scaled by 1/HID
    for mc in range(MC):
        nc.tensor.matmul(stats[0:1, :n], ones_inv,
                         h2[:, mc * ncols: mc * ncols + n],
                         start=(mc == 0), stop=(mc == MC - 1))
    for mc in range(MC):
        nc.tensor.matmul(stats[32:33, :n], ones_inv,
                         h2sq[:, mc * ncols: mc * ncols + n],
                         start=(mc == 0), stop=(mc == MC - 1))
    mu = sb.tile([1, n], F32, tag=f"{tag}_mu", bufs=1)
    nc.scalar.copy(mu, stats[0:1, :n])
    eh2 = sb.tile([1, n], F32, tag=f"{tag}_eh2", bufs=1)
    nc.scalar.copy(eh2, stats[32:33, :n])
    var = sb.tile([1, n], F32, tag=f"{tag}_var", bufs=1)
    nc.vector.tensor_mul(var, mu, mu)
    nc.vector.tensor_sub(var, eh2, var)
    u = sb.tile([1, n], F32, tag=f"{tag}_u", bufs=1)
    nc.scalar.activation(u, var, AF.Sqrt, bias=eps_t, scale=1.0)
    # B = pwg^T @ (h2) - c*mu + v*u   (columns of B = u[d] * z[d] with
    # z the true projected LN output; positive scale -> same direction)
    zd = sb.tile([128, KC2 * ncols], F32, tag=f"{tag}_zd", bufs=1)
    for mc in range(MC):
        b_ps = ps.tile([128, n], F32, tag=f"{tag}_bps", bufs=1)
        for kc in range(KC2):
            nc.tensor.matmul(b_ps,
                             pwg[:, kc * HID + mc * 128: kc * HID + (mc + 1) * 128],
                             h2[:, kc * ncols: kc * ncols + n],
                             start=(kc == 0), stop=False)
        nc.tensor.matmul(b_ps, c_neg[0:1, mc * 128:(mc + 1) * 128], mu,
                         start=False, stop=False)
        nc.tensor.matmul(b_ps, vrow[0:1, mc * 128:(mc + 1) * 128], u,
                         start=False, stop=True)
        nc.scalar.copy(zd[:, mc * ncols: mc * ncols + n], b_ps)
        zsq = sb.tile([128, n], F32, tag=f"{tag}_zsq", bufs=2)
        nc.scalar.square(zsq, b_ps)
        nc.tensor.matmul(stats[64:65, :n], ones_col, zsq,
                         start=(mc == 0), stop=(mc == MC - 1))
    nrm = sb.tile([1, n], F32, tag=f"{tag}_nrm", bufs=1)
    nc.scalar.activation(nrm, stats[64:65, :n], AF.Sqrt, bias=zero_t, scale=1.0)
    ninv = sb.tile([1, n], F32, tag=f"{tag}_ninv", bufs=1)
    scr = sb.tile([1, n], F32, tag=f"{tag}_scr", bufs=1)
    nc.vector.reciprocal_approx_accurate(ninv, nrm, scr)
    return zd, ninv, nrm


def build_nc():
    nc = bacc.Bacc(None, target_bir_lowering=False)

    docs_d = nc.dram_tensor("docs", [DPC, D_TOK, EMB], F32, kind="ExternalInput")
    q_d = nc.dram_tensor("q", [NQ, Q_TOK, EMB], F32, kind="ExternalInput")
    qW1_d = nc.dram_tensor("qW1", [KC1, 128, HID], F32, kind="ExternalInput")
    qW2_d = nc.dram_tensor("qW2", [KC2, 128, HID], F32, kind="ExternalInput")
    dW1_d = nc.dram_tensor("dW1", [KC1, 128, HID], F32, kind="ExternalInput")
    dW2_d = nc.dram_tensor("dW2", [KC2, 128, HID], F32, kind="ExternalInput")
    pW_d = nc.dram_tensor("pW", [KC2, 128, HID], F32, kind="ExternalInput")
    qb1_d = nc.dram_tensor("qb1", [MC, 128], F32, kind="ExternalInput")
    qb2_d = nc.dram_tensor("qb2", [MC, 128], F32, kind="ExternalInput")
    db1_d = nc.dram_tensor("db1", [MC, 128], F32, kind="ExternalInput")
    db2_d = nc.dram_tensor("db2", [MC, 128], F32, kind="ExternalInput")
    qg_d = nc.dram_tensor("qg", [MC, 128], F32, kind="ExternalInput")
    dg_d = nc.dram_tensor("dg", [MC, 128], F32, kind="ExternalInput")
    qbeta_d = nc.dram_tensor("qbeta", [MC, 128], F32, kind="ExternalInput")
    dbeta_d = nc.dram_tensor("dbeta", [MC, 128], F32, kind="ExternalInput")
    pb_d = nc.dram_tensor("pb", [1, HID], F32, kind="ExternalInput")

    o_val8 = nc.dram_tensor("o_val8", [128, 8], F32, kind="ExternalOutput")
    o_idx8 = nc.dram_tensor("o_idx8", [128, 8], U32, kind="ExternalOutput")
    o_qnrm = nc.dram_tensor("o_qnrm", [1, NQ], F32, kind="ExternalOutput")

    with TileContext(nc) as tc:
        with tc.tile_pool(name="persist", bufs=1) as pers:
            ident = pers.tile([128, 128], F32)
            make_identity(nc, ident)
            ones_col = pers.tile([128, 1], F32)
            nc.vector.memset(ones_col, 1.0)
            ones_inv = pers.tile([128, 1], F32)
            nc.vector.memset(ones_inv, 1.0 / HID)
            ones_row = pers.tile([1, 128], F32)
            nc.vector.memset(ones_row, 1.0)
            eps_t = pers.tile([1, 1], F32)
            nc.vector.memset(eps_t, LN_EPS)
            zero_t = pers.tile([1, 1], F32)
            nc.vector.memset(zero_t, 0.0)
            pb_row = pers.tile([1, HID], F32)
            nc.sync.dma_start(out=pb_row, in_=pb_d[:, :])

            # doc-encoder weights (persistent); pW staging tile is scoped
            with tc.tile_pool(name="p0tmp", bufs=1) as p0tmp:
                dW1s, dW2s, pw_t, pwg_d = _mlp_weights(tc, pers, nc, dW1_d,
                                                       dW2_d, pW_d, dg_d,
                                                       float(D_TOK),
                                                       tmp_sb=p0tmp)
                db1_t = pers.tile([128, MC], F32)
                nc.sync.dma_start(out=db1_t, in_=db1_d[:, :].rearrange("c p -> p c"))
                db2_t = pers.tile([128, MC], F32)
                nc.sync.dma_start(out=db2_t, in_=db2_d[:, :].rearrange("c p -> p c"))
                dbeta_t = pers.tile([128, MC], F32)
                nc.sync.dma_start(out=dbeta_t, in_=dbeta_d[:, :].rearrange("c p -> p c"))
                c_neg_d, vrow_d = _proj_consts(tc, pers, nc, pw_t, pwg_d,
                                               dbeta_t, pb_row, ones_col, "d")

            # query reps, duplicated across both M halves: [128, MC*128]
            qnT2 = pers.tile([128, MC * 128], F32)
            # per-query-row running scores [128, HI_W]
            scores = pers.tile([128, HI_W], F32)
            nc.vector.memset(scores[0:NQ, HI_OFF:HI_W], -1e30)

            # ---------------- Phase 1: encode queries ----------------
            with tc.tile_pool(name="qph", bufs=1) as qsb, \
                 tc.tile_pool(name="qph_ps", bufs=1, space="PSUM") as qps:
                qW1s, qW2s, qpw_t, pwg_q = _mlp_weights(tc, qsb, nc, qW1_d,
                                                        qW2_d, pW_d, qg_d,
                                                        float(Q_TOK))
                qb1_t = qsb.tile([128, MC], F32)
                nc.sync.dma_start(out=qb1_t, in_=qb1_d[:, :].rearrange("c p -> p c"))
                qb2_t = qsb.tile([128, MC], F32)
                nc.sync.dma_start(out=qb2_t, in_=qb2_d[:, :].rearrange("c p -> p c"))
                qbeta_t = qsb.tile([128, MC], F32)
                nc.sync.dma_start(out=qbeta_t, in_=qbeta_d[:, :].rearrange("c p -> p c"))
                c_neg_q, vrow_q = _proj_consts(tc, qsb, nc, qpw_t, pwg_q,
                                               qbeta_t, pb_row, ones_col, "q")

                # token-sum the queries: 2 half loads + tree
                qa = qsb.tile([NQ, (Q_TOK // 2) * EMB], F32)
                nc.sync.dma_start(out=qa.rearrange("p (t e) -> p t e", e=EMB),
                                  in_=q_d[:, 0:Q_TOK // 2, :])
                qb = qsb.tile([NQ, (Q_TOK // 2) * EMB], F32)
                nc.sync.dma_start(out=qb.rearrange("p (t e) -> p t e", e=EMB),
                                  in_=q_d[:, Q_TOK // 2:Q_TOK, :])
                # tree-sum in place to keep P1 SBUF small
                nc.vector.tensor_add(qa, qa, qb)            # 16 groups [12288]
                t6 = qb[:, :8 * EMB]
                nc.vector.tensor_add(t6, qa[:, :8 * EMB], qa[:, 8 * EMB:])
                t3 = qa[:, :4 * EMB]
                nc.vector.tensor_add(t3, t6[:, :4 * EMB], t6[:, 4 * EMB:])
                t15 = qb[:, :2 * EMB]
                nc.vector.tensor_add(t15, t3[:, :2 * EMB], t3[:, 2 * EMB:])
                xq = qa[:, :EMB]
                nc.vector.tensor_add(xq, t15[:, :EMB], t15[:, EMB:])

                # transpose to feature-major [128, KC1*64]
                xqT = qsb.tile([128, KC1 * NQ], F32)
                for ec in range(KC1):
                    q_tr = qps.tile([128, NQ], F32, tag="qtr", bufs=2)
                    nc.tensor.transpose(q_tr, xq[:, ec * 128:(ec + 1) * 128],
                                        ident[0:NQ, 0:NQ])
                    nc.scalar.copy(xqT[:, ec * NQ:(ec + 1) * NQ], q_tr)

                zq, ninv_q, nrm_q = _encode(tc, qsb, qps, nc, xqT, NQ, qW1s,
                                            qW2s, pwg_q, qb1_t, qb2_t,
                                            c_neg_q, vrow_q, ones_inv,
                                            ones_col, eps_t, zero_t, tag="q")
                # un-normalized reps; host divides candidate scores by qnrm
                nc.sync.dma_start(out=o_qnrm[:, :], in_=nrm_q)
                for mc in range(MC):
                    nc.scalar.copy(qnT2[:, mc * 128: mc * 128 + NQ],
                                   zq[:, mc * NQ:(mc + 1) * NQ])
                    nc.scalar.copy(qnT2[:, mc * 128 + NQ:(mc + 1) * 128],
                                   zq[:, mc * NQ:(mc + 1) * NQ])

            # ---------------- Phase 2: stream doc blocks ----------------
            with tc.tile_pool(name="main", bufs=1) as msb, \
                 tc.tile_pool(name="main_ps", bufs=1, space="PSUM") as mps:
                for mac in MACROS:
                    n = len(mac) * BLK
                    mac_start = BLOCK_STARTS[mac[0]]
                    xT = msb.tile([128, KC1 * MAC], F32, tag="xT", bufs=2)
                    for j, b in enumerate(mac):
                        s = BLOCK_STARTS[b]
                        qt = []
                        for t in range(4):
                            qq = msb.tile([BLK, 4 * EMB], F32, tag="quart",
                                          bufs=5)
                            nc.sync.dma_start(
                                out=qq.rearrange("p (t e) -> p t e", e=EMB),
                                in_=docs_d[s:s + BLK, 4 * t:4 * t + 4, :])
                            qt.append(qq)
                        nc.vector.tensor_add(qt[0], qt[0], qt[1])
                        nc.vector.tensor_add(qt[2], qt[2], qt[3])
                        nc.vector.tensor_add(qt[0], qt[0], qt[2])
                        nc.vector.tensor_add(qt[0][:, :2 * EMB],
                                             qt[0][:, :2 * EMB],
                                             qt[0][:, 2 * EMB:])
                        x768 = msb.tile([BLK, EMB], F32, tag="x768", bufs=2)
                        nc.vector.tensor_add(x768, qt[0][:, :EMB],
                                             qt[0][:, EMB:2 * EMB])
                        for ec in range(KC1):
                            d_tr = mps.tile([128, BLK], F32, tag="dtr", bufs=2)
                            nc.tensor.transpose(
                                d_tr, x768[:, ec * 128:(ec + 1) * 128], ident)
                            nc.scalar.copy(
                                xT[:, ec * MAC + j * BLK: ec * MAC + (j + 1) * BLK],
                                d_tr)

                    zd, ninv_d, _ = _encode(tc, msb, mps, nc, xT, n, dW1s,
                                            dW2s, pwg_d, db1_t, db2_t,
                                            c_neg_d, vrow_d, ones_inv,
                                            ones_col, eps_t, zero_t, tag="d")
                    # broadcast ninv over partitions (gpsimd; engines can't)
                    nb = msb.tile([128, n], F32, tag="nb", bufs=2)
                    nc.gpsimd.partition_broadcast(nb, ninv_d)
                    sc_ps = mps.tile([128, n], F32, tag="scps", bufs=1)
                    for mc in range(MC):
                        nc.tensor.matmul(sc_ps, qnT2[:, mc * 128:(mc + 1) * 128],
                                         zd[:, mc * MAC: mc * MAC + n],
                                         start=(mc == 0), stop=(mc == MC - 1))
                    if mac_start < HI_OFF:
                        lo, hi = 0, NQ
                        c0 = mac_start
                    else:
                        lo, hi = NQ, 128
                        c0 = mac_start - HI_OFF
                    nc.vector.tensor_mul(scores[lo:hi, c0:c0 + n],
                                         sc_ps[lo:hi, :], nb[lo:hi, :])

                # ---------------- Phase 3: local top-8 ----------------
                val8 = msb.tile([128, 8], F32)
                idx8 = msb.tile([128, 8], U32)
                nc.vector.max_with_indices(val8, idx8, scores)
                nc.sync.dma_start(out=o_val8[:, :], in_=val8)
                nc.sync.dma_start(out=o_idx8[:, :], in_=idx8)

    nc.finalize()
    return nc


_NC = None
LAST_RESULTS = None


def _install_ntff_hook():
    """The agent image's antenv lacks axon_hooks; shim it so trace=True works."""
    import sys
    import types
    if "antenv.axon_hooks" in sys.modules:
        return
    try:
        from trn_agent_boot.trn_boot import _ntff_profile_via_ctypes
        hook = _ntff_profile_via_ctypes("/opt/axon/libaxon_pjrt.so")
    except Exception:
        hook = None
    m = types.ModuleType("antenv.axon_hooks")
    m.get_axon_ntff_profile_hook = lambda: hook
    m.set_axon_ntff_profile_hook = lambda h: None
    sys.modules["antenv.axon_hooks"] = m


def _get_nc():
    global _NC
    if _NC is None:
        _NC = build_nc()
    return _NC


def kernel(query_embeddings, doc_embeddings, qW1, qb1, qW2, qb2, qg, qbeta,
           dW1, db1, dW2, db2, dg, dbeta, pW, pb, temperature):
    f = lambda a: np.ascontiguousarray(np.asarray(a), dtype=np.float32)
    docs = f(doc_embeddings)
    shared = {
        "q": f(query_embeddings),
        "qW1": f(qW1).reshape(KC1, 128, HID),
        "qW2": f(qW2).reshape(KC2, 128, HID),
        "dW1": f(dW1).reshape(KC1, 128, HID),
        "dW2": f(dW2).reshape(KC2, 128, HID),
        "pW": f(pW).reshape(KC2, 128, HID),
        "qb1": f(qb1).reshape(MC, 128),
        "qb2": f(qb2).reshape(MC, 128),
        "db1": f(db1).reshape(MC, 128),
        "db2": f(db2).reshape(MC, 128),
        "qg": f(qg).reshape(MC, 128),
        "dg": f(dg).reshape(MC, 128),
        "qbeta": f(qbeta).reshape(MC, 128),
        "dbeta": f(dbeta).reshape(MC, 128),
        "pb": f(pb).reshape(1, HID),
    }
    in_maps = [dict(shared, docs=docs[c * DPC:(c + 1) * DPC]) for c in range(N_CORES)]
    nc = _get_nc()
    trace = bool(os.environ.get("KERNEL_TRACE"))
    if trace:
        _install_ntff_hook()
    res = run_bass_kernel_spmd(nc, in_maps, core_ids=list(range(N_CORES)),
                               trace=trace)
    global LAST_RESULTS
    LAST_RESULTS = res
    if trace and res.exec_time_ns is not None:
        print(f"HW exec time: {res.exec_time_ns} ns")
        if res.instructions_and_trace:
            print(f"trace: {res.instructions_and_trace[1]}")

    vals = np.stack([r["o_val8"] for r in res.results])          # [8, 128, 8]
    idxs = np.stack([r["o_idx8"].astype(np.int64) for r in res.results])
    qnrm = res.results[0]["o_qnrm"][0]                           # [64]

    # flatten candidates: core c, half h, slot k
    cand_v = np.concatenate(
        [vals[c, h * NQ:(h + 1) * NQ, :] for c in range(N_CORES) for h in (0, 1)],
        axis=1)                                                  # [64, 128]
    cand_i = np.concatenate(
        [idxs[c, h * NQ:(h + 1) * NQ, :] + c * DPC + h * HI_OFF
         for c in range(N_CORES) for h in (0, 1)], axis=1)       # [64, 128]

    temp = np.float32(np.asarray(temperature))
    cand_s = (cand_v / qnrm[:, None].astype(np.float32)) / temp  # f32 [64, 128]

    k = 8
    top_scores = np.empty((NQ, k), np.float32)
    top_idx = np.empty((NQ, k), np.int64)
    for qi in range(NQ):
        order = np.lexsort((cand_i[qi], -cand_s[qi].astype(np.float64)))[:k]
        top_scores[qi] = cand_s[qi][order]
        top_idx[qi] = cand_i[qi][order]

    retrieved = docs[top_idx]                                    # [64, 8, 16, 768]
    return top_scores, retrieved


# revision 13
# speedup vs baseline: 1.0516x; 1.0516x over previous
"""NeuralRetriever kernel for 8x Trainium2 NeuronCores (Bass/Tile).

Full inputs in, full outputs out. Internally: doc_embeddings sharded along
num_docs across 8 cores; each core mean-pools + encodes its shard, scores all
64 queries against it, and returns a local top-8 (values + indices) per
query-half-row. Host merges the 8*2*8 candidates per query and re-top-ks.

reference semantics (see problem):
    q = mlp_ln(mean_t(query_embeddings)); d = mlp_ln(mean_t(doc_embeddings))
    qn = l2norm(q @ pW + pb); dn = l2norm(d @ pW + pb)
    scores = qn @ dn.T / temperature; top8 + gathered docs.

Layout notes:
  - everything after pooling is feature-major ([feature, doc] with features on
    partitions), so all contractions have K on partitions and the only
    transposes are 6 PE transposes per 128-doc block of the pooled input.
  - LN is folded: with u = sqrt(var+eps), B := u * z = pwg^T H2 - c x mu + v x u
    (c = colsums of pwg, v = pW^T beta + pb). l2norm cancels the u scale, so
    d_norm = B / ||B||. The two rank-1 corrections are one K=33 matmul with
    c/v stacked at partitions 0/32 (matmul output bases must be 0/32/64).
  - scores: query reps are NOT normalized on device; host divides by o_qnrm.
    1/temperature is applied on host as well.
"""

import os

import numpy as np

import concourse.bass as bass
import concourse.mybir as mybir
from concourse import bacc
from concourse.tile import TileContext
from concourse.bass_utils import run_bass_kernel_spmd
from concourse.masks import make_identity

F32 = mybir.dt.float32
U32 = mybir.dt.uint32
AF = mybir.ActivationFunctionType

N_CORES = 8
N_DOCS = 50000
DPC = N_DOCS // N_CORES          # 6250 docs per core
D_TOK = 16
Q_TOK = 32
EMB = 768
HID = 512
NQ = 64
LN_EPS = 1e-5

BLK = 128                         # pooling block (docs)
MAC = 512                         # matmul macro-block (docs)
HI_OFF = 3072                     # docs >= HI_OFF go to partition 64+q
HI_W = DPC - HI_OFF               # 3178 columns in the hi half
KC1 = EMB // 128                  # 6  K-chunks for layer 1
KC2 = HID // 128                  # 4  K-chunks for hid-dim contractions
MC = HID // 128                   # 4  M-chunks of hidden dim

# pooling blocks: 48 aligned + 1 overlapping tail covering [6122, 6250)
BLOCK_STARTS = [i * BLK for i in range(DPC // BLK)] + [DPC - BLK]
# macros: groups of 4 blocks (512 docs) + tail macro of 1 block
MACROS = [list(range(i, i + 4)) for i in range(0, 48, 4)] + [[48]]


def _mlp_weights(tc, sb, nc, w1_d, w2_d, pw_d, g_d, tok_div):
    """Load + prep encoder weights into SBUF lhsT layouts.

    Returns (w1s [128, KC1*512] scaled by 1/tok_div, w2s [128, KC2*512],
             pwg [128, KC2*512]). pwg is loaded as pW and scaled in place by
    g[k] AFTER callers compute v from it (Tile's WAR dep keeps the order)."""
    w1s = sb.tile([128, KC1 * HID], F32, name=f"w1s_{tok_div}")
    nc.sync.dma_start(out=w1s.rearrange("p (k m) -> p k m", k=KC1),
                      in_=w1_d[:, :, :].rearrange("k p m -> p k m"))
    nc.vector.tensor_scalar_mul(w1s, w1s, 1.0 / tok_div)
    w2s = sb.tile([128, KC2 * HID], F32, name=f"w2s_{tok_div}")
    nc.sync.dma_start(out=w2s.rearrange("p (k m) -> p k m", k=KC2),
                      in_=w2_d[:, :, :].rearrange("k p m -> p k m"))
    g_t = sb.tile([128, KC2], F32, name=f"g_{tok_div}")
    nc.sync.dma_start(out=g_t, in_=g_d[:, :].rearrange("c p -> p c"))
    pwg = sb.tile([128, KC2 * HID], F32, name=f"pwg_{tok_div}")
    nc.sync.dma_start(out=pwg.rearrange("p (k m) -> p k m", k=KC2),
                      in_=pw_d[:, :, :].rearrange("k p m -> p k m"))
    return w1s, w2s, g_t, pwg


def _proj_consts(tc, sb, nc, pwg, g_t, beta_t, pb_row, ones_col, tag,
                 ps=None):
    """cv [33, 512] SBUF: partition 0 = -colsum(pW*g), partition 32 =
    pW^T beta + pb. Consumes pwg while it still holds plain pW, then scales
    it in place by g."""
    import contextlib
    ctx = (tc.tile_pool(name=f"{tag}_cps", bufs=1, space="PSUM")
           if ps is None else contextlib.nullcontext(ps))
    with ctx as ps:
        cv_ps = ps.tile([33, HID], F32, name=f"cvps_{tag}", tag=f"{tag}cv")
        # v first (needs un-scaled pW): v = pW^T beta + pb  -> partition 32
        for kc in range(KC2):
            nc.tensor.matmul(cv_ps[32:33, :], beta_t[:, kc:kc + 1],
                             pwg[:, kc * HID:(kc + 1) * HID],
                             start=(kc == 0), stop=False)
        nc.tensor.matmul(cv_ps[32:33, :], ones_col[0:1, 0:1], pb_row,
                         start=False, stop=True)
        # scale pwg in place by per-partition g
        for kc in range(KC2):
            nc.vector.tensor_scalar(out=pwg[:, kc * HID:(kc + 1) * HID],
                                    in0=pwg[:, kc * HID:(kc + 1) * HID],
                                    scalar1=g_t[:, kc:kc + 1], scalar2=None,
                                    op0=mybir.AluOpType.mult)
        # c = colsum(pwg) -> partition 0
        for kc in range(KC2):
            nc.tensor.matmul(cv_ps[0:1, :], ones_col,
                             pwg[:, kc * HID:(kc + 1) * HID],
                             start=(kc == 0), stop=(kc == KC2 - 1))
        cv = sb.tile([33, HID], F32, name=f"cv_{tag}")
        nc.vector.memset(cv, 0.0)
        nc.scalar.mul(cv[0:1, :], cv_ps[0:1, :], -1.0)
        nc.scalar.copy(cv[32:33, :], cv_ps[32:33, :])
    return cv


def _encode(tc, sb, ps, nc, xT, n, w1s, w2s, pwg, b1_t, b2_t, cv,
            ones_inv, ones_col, eps_t, zero33, tag, share_ps=False):
    """Feature-major MLP + LN + proj with LN scale folded (B = A + u*v).

    xT: SBUF tile [128, KC1*ncols], chunk kc in cols [kc*ncols, kc*ncols+n).
    Returns (zd [128, KC2*ncols] un-normalized projected reps,
             ninv33 [33, n] with 1/||B|| at partition 32,
             nrm33 [33, n] with ||B|| at partition 32)."""
    ncols = xT.shape[1] // KC1
    h1 = sb.tile([128, KC2 * ncols], F32, tag=f"{tag}_h1", bufs=1)
    for mc in range(MC):
        h1_ps = ps.tile([128, n], F32, tag=(f"{tag}_mmps" if share_ps else f"{tag}_h1ps"), bufs=1)
        for kc in range(KC1):
            nc.tensor.matmul(h1_ps,
                             w1s[:, kc * HID + mc * 128: kc * HID + (mc + 1) * 128],
                             xT[:, kc * ncols: kc * ncols + n],
                             start=(kc == 0), stop=(kc == KC1 - 1))
        nc.scalar.activation(h1[:, mc * ncols: mc * ncols + n], h1_ps,
                             AF.Relu, bias=b1_t[:, mc:mc + 1], scale=1.0)
    h2 = sb.tile([128, KC2 * ncols], F32, tag=f"{tag}_h2", bufs=1)
    h2sq = sb.tile([128, KC2 * ncols], F32, tag=f"{tag}_h2sq", bufs=1)
    # stats_a: mu at partition 0 (group 1), ||B||^2 at partition 32 (group 2)
    # stats_b: E[h^2] at partition 0
    stats_a = ps.tile([33, n], F32, tag=f"{tag}_statsa", bufs=1)
    stats_b = ps.tile([1, n], F32, tag=f"{tag}_statsb", bufs=1)
    for mc in range(MC):
        h2_ps = ps.tile([128, n], F32, tag=(f"{tag}_mmps" if share_ps else f"{tag}_h2ps"), bufs=1)
        for kc in range(KC2):
            nc.tensor.matmul(h2_ps,
                             w2s[:, kc * HID + mc * 128: kc * HID + (mc + 1) * 128],
                             h1[:, kc * ncols: kc * ncols + n],
                             start=(kc == 0), stop=(kc == KC2 - 1))
        nc.scalar.activation(h2[:, mc * ncols: mc * ncols + n], h2_ps,
                             AF.Identity, bias=b2_t[:, mc:mc + 1], scale=1.0)
        nc.scalar.activation(h2sq[:, mc * ncols: mc * ncols + n], h2_ps,
                             AF.Square, bias=b2_t[:, mc:mc + 1], scale=1.0)
    for mc in range(MC):
        nc.tensor.matmul(stats_a[0:1, :n], ones_inv,
                         h2[:, mc * ncols: mc * ncols + n],
                         start=(mc == 0), stop=(mc == MC - 1))
    for mc in range(MC):
        nc.tensor.matmul(stats_b[0:1, :n], ones_inv,
                         h2sq[:, mc * ncols: mc * ncols + n],
                         start=(mc == 0), stop=(mc == MC - 1))
    # muu [33, n]: mu at partition 0, u = sqrt(var+eps) at partition 32
    muu = sb.tile([33, n], F32, tag=f"{tag}_muu", bufs=1)
    nc.scalar.copy(muu[0:1, :], stats_a[0:1, :n])
    eh2 = sb.tile([1, n], F32, tag=f"{tag}_eh2", bufs=1)
    nc.scalar.copy(eh2, stats_b[0:1, :n])
    var = sb.tile([1, n], F32, tag=f"{tag}_var", bufs=1)
    nc.vector.tensor_mul(var, muu[0:1, :], muu[0:1, :])
    nc.vector.tensor_sub(var, eh2, var)
    u = sb.tile([1, n], F32, tag=f"{tag}_u", bufs=1)
    nc.scalar.activation(u, var, AF.Sqrt, bias=eps_t, scale=1.0)
    nc.sync.dma_start(out=muu[32:33, :], in_=u)       # partition hop 0 -> 32
    zd = sb.tile([128, KC2 * ncols], F32, tag=f"{tag}_zd", bufs=1)
    for mc in range(MC):
        b_ps = ps.tile([128, n], F32, tag=(f"{tag}_mmps" if share_ps else f"{tag}_bps"), bufs=1)
        for kc in range(KC2):
            nc.tensor.matmul(b_ps,
                             pwg[:, kc * HID + mc * 128: kc * HID + (mc + 1) * 128],
                             h2[:, kc * ncols: kc * ncols + n],
                             start=(kc == 0), stop=False)
        nc.tensor.matmul(b_ps, cv[:, mc * 128:(mc + 1) * 128], muu,
                         start=False, stop=True)
        nc.scalar.copy(zd[:, mc * ncols: mc * ncols + n], b_ps)
        zsq = sb.tile([128, n], F32, tag=f"{tag}_zsq", bufs=2)
        nc.scalar.square(zsq, b_ps)
        nc.tensor.matmul(stats_a[32:33, :n], ones_col, zsq,
                         start=(mc == 0), stop=(mc == MC - 1))
    # hop ||B||^2 from psum partition 32 down to partition 0, then finish
    ssz32 = sb.tile([33, n], F32, tag=f"{tag}_ssz32", bufs=1)
    nc.scalar.copy(ssz32[32:33, :], stats_a[32:33, :n])
    ssz0 = sb.tile([1, n], F32, tag=f"{tag}_ssz0", bufs=1)
    nc.sync.dma_start(out=ssz0, in_=ssz32[32:33, :])
    nrm = sb.tile([1, n], F32, tag=f"{tag}_nrm", bufs=1)
    nc.scalar.activation(nrm, ssz0, AF.Sqrt, bias=zero33[0:1, :], scale=1.0)
    ninv = sb.tile([1, n], F32, tag=f"{tag}_ninv", bufs=1)
    scr = sb.tile([1, n], F32, tag=f"{tag}_scr", bufs=1)
    nc.vector.reciprocal_approx_accurate(ninv, nrm, scr)
    return zd, ninv, nrm


def build_nc():
    nc = bacc.Bacc(None, target_bir_lowering=False)

    docs_d = nc.dram_tensor("docs", [DPC, D_TOK, EMB], F32, kind="ExternalInput")
    q_d = nc.dram_tensor("q", [NQ, Q_TOK, EMB], F32, kind="ExternalInput")
    qW1_d = nc.dram_tensor("qW1", [KC1, 128, HID], F32, kind="ExternalInput")
    qW2_d = nc.dram_tensor("qW2", [KC2, 128, HID], F32, kind="ExternalInput")
    dW1_d = nc.dram_tensor("dW1", [KC1, 128, HID], F32, kind="ExternalInput")
    dW2_d = nc.dram_tensor("dW2", [KC2, 128, HID], F32, kind="ExternalInput")
    pW_d = nc.dram_tensor("pW", [KC2, 128, HID], F32, kind="ExternalInput")
    qb1_d = nc.dram_tensor("qb1", [MC, 128], F32, kind="ExternalInput")
    qb2_d = nc.dram_tensor("qb2", [MC, 128], F32, kind="ExternalInput")
    db1_d = nc.dram_tensor("db1", [MC, 128], F32, kind="ExternalInput")
    db2_d = nc.dram_tensor("db2", [MC, 128], F32, kind="ExternalInput")
    qg_d = nc.dram_tensor("qg", [MC, 128], F32, kind="ExternalInput")
    dg_d = nc.dram_tensor("dg", [MC, 128], F32, kind="ExternalInput")
    qbeta_d = nc.dram_tensor("qbeta", [MC, 128], F32, kind="ExternalInput")
    dbeta_d = nc.dram_tensor("dbeta", [MC, 128], F32, kind="ExternalInput")
    pb_d = nc.dram_tensor("pb", [1, HID], F32, kind="ExternalInput")

    o_val8 = nc.dram_tensor("o_val8", [128, 8], F32, kind="ExternalOutput")
    o_idx8 = nc.dram_tensor("o_idx8", [128, 8], U32, kind="ExternalOutput")
    o_qnrm = nc.dram_tensor("o_qnrm", [1, NQ], F32, kind="ExternalOutput")

    with TileContext(nc) as tc:
        with tc.tile_pool(name="persist", bufs=1) as pers:
            ident = pers.tile([128, 128], F32)
            make_identity(nc, ident)
            ones_col = pers.tile([128, 1], F32)
            nc.vector.memset(ones_col, 1.0)
            ones_inv = pers.tile([128, 1], F32)
            nc.vector.memset(ones_inv, 1.0 / HID)
            eps_t = pers.tile([1, 1], F32)
            nc.vector.memset(eps_t, LN_EPS)
            zero33 = pers.tile([33, 1], F32)
            nc.vector.memset(zero33, 0.0)
            pb_row = pers.tile([1, HID], F32)
            nc.sync.dma_start(out=pb_row, in_=pb_d[:, :])

            # doc-encoder weights (persistent)
            dW1s, dW2s, dg_t, pwg_d = _mlp_weights(tc, pers, nc, dW1_d, dW2_d,
                                                   pW_d, dg_d, float(D_TOK))
            db1_t = pers.tile([128, MC], F32)
            nc.sync.dma_start(out=db1_t, in_=db1_d[:, :].rearrange("c p -> p c"))
            db2_t = pers.tile([128, MC], F32)
            nc.sync.dma_start(out=db2_t, in_=db2_d[:, :].rearrange("c p -> p c"))
            dbeta_t = pers.tile([128, MC], F32)
            nc.sync.dma_start(out=dbeta_t, in_=dbeta_d[:, :].rearrange("c p -> p c"))
            cv_d = _proj_consts(tc, pers, nc, pwg_d, dg_t, dbeta_t, pb_row,
                                ones_col, "d")

            # query reps, duplicated across both M halves: [128, MC*128]
            qnT2 = pers.tile([128, MC * 128], F32)
            # per-query-row running scores [128, HI_W]
            scores = pers.tile([128, HI_W], F32)
            nc.vector.memset(scores[0:NQ, HI_OFF:HI_W], -1e30)

            # doc streaming pools: opened BEFORE the query-phase pool so their
            # SBUF/PSUM addresses don't overlap it -> doc DMA + pooling +
            # transposes run concurrently with query encode.
            with tc.tile_pool(name="docio", bufs=1) as dio, \
                 tc.tile_pool(name="docio_ps", bufs=1, space="PSUM") as diops:

                # ---------- emit doc DMA/pool/transpose closures ----------
                def emit_block(b, xT):
                    j = b % 4
                    s = BLOCK_STARTS[b]
                    qt = []
                    for t in range(4):
                        qq = dio.tile([BLK, 4 * EMB], F32, tag="quart", bufs=4,
                                      name=f"qq_{b}_{t}")
                        nc.sync.dma_start(
                            out=qq.rearrange("p (t e) -> p t e", e=EMB),
                            in_=docs_d[s:s + BLK, 4 * t:4 * t + 4, :])
                        qt.append(qq)
                    nc.vector.tensor_add(qt[0], qt[0], qt[1])
                    nc.vector.tensor_add(qt[2], qt[2], qt[3])
                    nc.vector.tensor_add(qt[0], qt[0], qt[2])
                    nc.vector.tensor_add(qt[0][:, :2 * EMB],
                                         qt[0][:, :2 * EMB],
                                         qt[0][:, 2 * EMB:])
                    x768 = dio.tile([BLK, EMB], F32, tag="x768", bufs=2,
                                    name=f"x768_{b}")
                    nc.vector.tensor_add(x768, qt[0][:, :EMB],
                                         qt[0][:, EMB:2 * EMB])
                    for ec in range(KC1):
                        d_tr = diops.tile([128, BLK], F32, tag="dtr", bufs=2,
                                          name=f"dtr_{b}_{ec}")
                        nc.tensor.transpose(
                            d_tr, x768[:, ec * 128:(ec + 1) * 128], ident)
                        nc.scalar.copy(
                            xT[:, ec * MAC + j * BLK: ec * MAC + (j + 1) * BLK],
                            d_tr)

                # ---------------- Phase 1: encode queries ----------------
                with tc.tile_pool(name="qph", bufs=1) as qsb, \
                     tc.tile_pool(name="qph_ps", bufs=1, space="PSUM") as qps:
                    qW1s, qW2s, qg_t, pwg_q = _mlp_weights(tc, qsb, nc, qW1_d,
                                                           qW2_d, pW_d, qg_d,
                                                           float(Q_TOK))
                    qb1_t = qsb.tile([128, MC], F32)
                    nc.sync.dma_start(out=qb1_t,
                                      in_=qb1_d[:, :].rearrange("c p -> p c"))
                    qb2_t = qsb.tile([128, MC], F32)
                    nc.sync.dma_start(out=qb2_t,
                                      in_=qb2_d[:, :].rearrange("c p -> p c"))
                    qbeta_t = qsb.tile([128, MC], F32)
                    nc.sync.dma_start(out=qbeta_t,
                                      in_=qbeta_d[:, :].rearrange("c p -> p c"))
                    cv_q = _proj_consts(tc, qsb, nc, pwg_q, qg_t, qbeta_t,
                                        pb_row, ones_col, "q", ps=qps)

                    # token-sum queries: 8 chunk loads accumulated in place
                    acc = qsb.tile([NQ, 4 * EMB], F32)
                    nc.sync.dma_start(
                        out=acc.rearrange("p (t e) -> p t e", e=EMB),
                        in_=q_d[:, 0:4, :])
                    for ch in range(1, 8):
                        qtmp = qsb.tile([NQ, 4 * EMB], F32, tag="qstg", bufs=2,
                                        name=f"qstg_{ch}")
                        nc.sync.dma_start(
                            out=qtmp.rearrange("p (t e) -> p t e", e=EMB),
                            in_=q_d[:, 4 * ch:4 * ch + 4, :])
                        nc.vector.tensor_add(acc, acc, qtmp)
                    nc.vector.tensor_add(acc[:, :2 * EMB], acc[:, :2 * EMB],
                                         acc[:, 2 * EMB:])
                    xq = acc[:, :EMB]
                    nc.vector.tensor_add(xq, acc[:, :EMB], acc[:, EMB:2 * EMB])

                    # transpose to feature-major [128, KC1*64]
                    xqT = qsb.tile([128, KC1 * NQ], F32)
                    for ec in range(KC1):
                        q_tr = qps.tile([128, NQ], F32, tag="qtr", bufs=1,
                                        name=f"qtr_{ec}")
                        nc.tensor.transpose(q_tr, xq[:, ec * 128:(ec + 1) * 128],
                                            ident[0:NQ, 0:NQ])
                        nc.scalar.copy(xqT[:, ec * NQ:(ec + 1) * NQ], q_tr)

                    zq, _, nrm_q = _encode(tc, qsb, qps, nc, xqT, NQ, qW1s,
                                           qW2s, pwg_q, qb1_t, qb2_t, cv_q,
                                           ones_inv, ones_col, eps_t, zero33,
                                           tag="q", share_ps=True)
                    # un-normalized reps; host divides by these norms
                    nc.sync.dma_start(out=o_qnrm[:, :], in_=nrm_q)
                    for mc in range(MC):
                        nc.scalar.copy(qnT2[:, mc * 128: mc * 128 + NQ],
                                       zq[:, mc * NQ:(mc + 1) * NQ])
                        nc.scalar.copy(qnT2[:, mc * 128 + NQ:(mc + 1) * 128],
                                       zq[:, mc * NQ:(mc + 1) * NQ])

                # ------------- Phase 2: doc macro loop -------------
                with tc.tile_pool(name="main", bufs=1) as msb, \
                     tc.tile_pool(name="main_ps", bufs=1, space="PSUM") as mps:
                    for mac in MACROS:
                        n = len(mac) * BLK
                        mac_start = BLOCK_STARTS[mac[0]]
                        xT = dio.tile([128, KC1 * MAC], F32, tag="xT", bufs=2,
                                      name=f"xT_{mac[0]}")
                        for b in mac:
                            emit_block(b, xT)

                        zd, ninv_d, _ = _encode(tc, msb, mps, nc, xT, n, dW1s,
                                                dW2s, pwg_d, db1_t, db2_t,
                                                cv_d, ones_inv, ones_col,
                                                eps_t, zero33, tag="d")
                        # broadcast 1/||B|| over partitions (gpsimd)
                        nb = msb.tile([128, n], F32, tag="nb", bufs=2,
                                      name=f"nb_{mac[0]}")
                        nc.gpsimd.partition_broadcast(nb, ninv_d)
                        sc_ps = mps.tile([128, n], F32, tag="scps", bufs=1,
                                         name=f"scps_{mac[0]}")
                        for mc in range(MC):
                            nc.tensor.matmul(sc_ps,
                                             qnT2[:, mc * 128:(mc + 1) * 128],
                                             zd[:, mc * MAC: mc * MAC + n],
                                             start=(mc == 0),
                                             stop=(mc == MC - 1))
                        if mac_start < HI_OFF:
                            lo, hi = 0, NQ
                            c0 = mac_start
                        else:
                            lo, hi = NQ, 128
                            c0 = mac_start - HI_OFF
                        nc.vector.tensor_mul(scores[lo:hi, c0:c0 + n],
                                             sc_ps[lo:hi, :], nb[lo:hi, :])

                    # ---------------- Phase 3: local top-8 ----------------
                    val8 = msb.tile([128, 8], F32)
                    idx8 = msb.tile([128, 8], U32)
                    nc.vector.max_with_indices(val8, idx8, scores)
                    nc.sync.dma_start(out=o_val8[:, :], in_=val8)
                    nc.sync.dma_start(out=o_idx8[:, :], in_=idx8)

    nc.finalize()
    return nc


_NC = None
LAST_RESULTS = None


def _install_ntff_hook():
    """The agent image's antenv lacks axon_hooks; shim it so trace=True works."""
    import sys
    import types
    if "antenv.axon_hooks" in sys.modules:
        return
    try:
        from trn_agent_boot.trn_boot import _ntff_profile_via_ctypes
        hook = _ntff_profile_via_ctypes("/opt/axon/libaxon_pjrt.so")
    except Exception:
        hook = None
    m = types.ModuleType("antenv.axon_hooks")
    m.get_axon_ntff_profile_hook = lambda: hook
    m.set_axon_ntff_profile_hook = lambda h: None
    sys.modules["antenv.axon_hooks"] = m


def _get_nc():
    global _NC
    if _NC is None:
        _NC = build_nc()
    return _NC


def kernel(query_embeddings, doc_embeddings, qW1, qb1, qW2, qb2, qg, qbeta,
           dW1, db1, dW2, db2, dg, dbeta, pW, pb, temperature):
    f = lambda a: np.ascontiguousarray(np.asarray(a), dtype=np.float32)
    docs = f(doc_embeddings)
    shared = {
        "q": f(query_embeddings),
        "qW1": f(qW1).reshape(KC1, 128, HID),
        "qW2": f(qW2).reshape(KC2, 128, HID),
        "dW1": f(dW1).reshape(KC1, 128, HID),
        "dW2": f(dW2).reshape(KC2, 128, HID),
        "pW": f(pW).reshape(KC2, 128, HID),
        "qb1": f(qb1).reshape(MC, 128),
        "qb2": f(qb2).reshape(MC, 128),
        "db1": f(db1).reshape(MC, 128),
        "db2": f(db2).reshape(MC, 128),
        "qg": f(qg).reshape(MC, 128),
        "dg": f(dg).reshape(MC, 128),
        "qbeta": f(qbeta).reshape(MC, 128),
        "dbeta": f(dbeta).reshape(MC, 128),
        "pb": f(pb).reshape(1, HID),
    }
    in_maps = [dict(shared, docs=docs[c * DPC:(c + 1) * DPC]) for c in range(N_CORES)]
    nc = _get_nc()
    trace = bool(os.environ.get("KERNEL_TRACE"))
    if trace:
        _install_ntff_hook()
    res = run_bass_kernel_spmd(nc, in_maps, core_ids=list(range(N_CORES)),
                               trace=trace)
    global LAST_RESULTS
    LAST_RESULTS = res
    if trace and res.exec_time_ns is not None:
        print(f"HW exec time: {res.exec_time_ns} ns")
        if res.instructions_and_trace:
            print(f"trace: {res.instructions_and_trace[1]}")

    vals = np.stack([r["o_val8"] for r in res.results])          # [8, 128, 8]
    idxs = np.stack([r["o_idx8"].astype(np.int64) for r in res.results])
    qnrm = res.results[0]["o_qnrm"][0]                           # [64]

    cand_v = np.concatenate(
        [vals[c, h * NQ:(h + 1) * NQ, :] for c in range(N_CORES) for h in (0, 1)],
        axis=1)                                                  # [64, 128]
    cand_i = np.concatenate(
        [idxs[c, h * NQ:(h + 1) * NQ, :] + c * DPC + h * HI_OFF
         for c in range(N_CORES) for h in (0, 1)], axis=1)       # [64, 128]

    temp = np.float32(np.asarray(temperature))
    cand_s = (cand_v / qnrm[:, None].astype(np.float32)) / temp  # f32 [64, 128]

    k = 8
    top_scores = np.empty((NQ, k), np.float32)
    top_idx = np.empty((NQ, k), np.int64)
    for qi in range(NQ):
        order = np.lexsort((cand_i[qi], -cand_s[qi].astype(np.float64)))[:k]
        top_scores[qi] = cand_s[qi][order]
        top_idx[qi] = cand_i[qi][order]

    retrieved = docs[top_idx]                                    # [64, 8, 16, 768]
    return top_scores, retrieved
